# revision 3
# baseline (speedup 1.0000x reference)
"""Trainium2 Bass kernel for nn_LoRALinear (DoRA-style LoRA linear).

Reference math:
    base = x @ W^T
    lora = sc * (x @ A^T) @ B^T          (sc = 2.0)
    w_eff = W + sc * (B @ A)
    s = magnitude / ||w_eff||_row
    out = base + (s - 1) * base + s * lora = x @ (s[:, None] * w_eff)^T

The whole op collapses to one dense matmul with a derived weight computed
host-side in fp32 during input prep, so the device kernel is a pure
streaming GEMM: per core [4096, 1024] @ [1024, 1024] in bf16 (fp32 PSUM),
which is PE-bound at the bf16 roofline (216 ns per 128x128x512 matmul,
110.6 us of matmul per core).

v2 over the 130 us baseline attacks the ~20 us of non-matmul time found
in the trace (5.9 us fixed NEFF entry, serial ~650 ns DMA triggers pacing
the startup fill to ~246 GB/s, first MM at 10.6 us, PE cold at 1.2 GHz
until 15.5 us, 5.9 us drain tail):
  - PE warm-up: ~32 dummy 32x128x128 matmuls on memset data issued right
    after the entry barrier keep the PE busy through the DMA fill window
    so the HAM clock gate reaches 2.4 GHz before real matmuls start.
  - x is host-repacked per chunk ([128 part, 8k x 256 tok] contiguous per
    256-token chunk) so each chunk is ONE 512 KB DMA trigger (128 x 4 KB
    descriptors) instead of 8; triggers stop pacing the stream.
  - Startup triggers spread across the queues that finish the entry
    barrier earliest: vector=x0, gpsimd=w0..w7, scalar=x1; sync carries
    the 14 steady-state x triggers. Weights + first two x chunks stream
    concurrently at full ring bandwidth from ~6.5 us.
  - Drain split: per (chunk, j) the two PSUM halves are copied by ACT and
    DVE as before, but each half gets its own 128 KB out-DMA (scalar for
    h=0, sync for h=1), halving the serialized tail after the last MM.
"""

import os
import numpy as np
from contextlib import ExitStack

import ml_dtypes

import concourse.bass as bass
import concourse.mybir as mybir
import concourse.tile as tile
from concourse import bacc
from concourse.bass import ts
from concourse.bass_utils import run_bass_kernel_spmd

N_CORES = 8
B, S, D_IN, D_OUT, R = 4, 8192, 1024, 1024, 16
SCALING = 32.0 / 16.0
M_TOT = B * S
M_CORE = M_TOT // N_CORES
P = 128
K_TILES = D_IN // P
CHUNK = 256
N_CHUNKS = M_CORE // CHUNK
SUB = CHUNK // P
NH = D_OUT // 512
XROW = K_TILES * CHUNK  # 2048 bf16 per partition per chunk
N_WARM = 32
F32 = mybir.dt.float32
BF16 = mybir.dt.bfloat16
BF16_NP = np.dtype(ml_dtypes.bfloat16)


def _kernel_body(ctx: ExitStack, tc: "tile.TileContext", xC, wsT, out):
    nc = tc.nc
    w_pool = ctx.enter_context(tc.tile_pool(name="w", bufs=1))
    x_pool = ctx.enter_context(tc.tile_pool(name="x", bufs=2))
    o_pool = ctx.enter_context(tc.tile_pool(name="o", bufs=4))
    ps_pool = ctx.enter_context(tc.tile_pool(name="ps", bufs=2, space="PSUM"))

    # --- startup: PE warm-up fodder + startup DMAs on the early queues ---
    warm = w_pool.tile([P, P], BF16, tag="warm", name="warm")
    nc.vector.memset(warm[:], 0.5)

    # x chunk 0 on the scalar queue (only gpsimd/sync/scalar can trigger
    # DMAs; scalar's entry barrier ends ~1.3 us before sync's), x chunk 1
    # on sync; both stream while gpsimd streams the weights.
    x0 = x_pool.tile([P, XROW], BF16, tag="x", name="x_0")
    nc.scalar.dma_start(x0[:], xC[ts(0, P), :])
    ws = []
    for k in range(K_TILES):
        w = w_pool.tile([P, D_OUT], BF16, tag=f"w{k}", name=f"w{k}")
        nc.gpsimd.dma_start(w[:], wsT[ts(k, P), :])
        ws.append(w)
    x1 = x_pool.tile([P, XROW], BF16, tag="x", name="x_1")
    nc.sync.dma_start(x1[:], xC[ts(1, P), :])

    # Dummy matmuls: keep the PE busy from ~5.9 us (end of entry barrier)
    # through the DMA fill window so the HAM clock gate un-throttles
    # (needs ~3.4 us of sustained busy) before the real matmuls begin.
    # They overwrite (start=True) a PSUM tile instance that chunk 1's
    # accumulation later reuses, long after the dummies retire.
    warm_ps = ps_pool.tile([P, 512], F32, tag="ps00", name="warm_ps")
    for i in range(N_WARM):
        nc.tensor.matmul(
            warm_ps[0:32, 0:128],
            lhsT=warm[:, 0:32],
            rhs=warm[:, :],
            start=True,
            stop=True,
        )

    xts = [x0, x1]
    for c in range(N_CHUNKS):
        if c >= 2:
            xt = x_pool.tile([P, XROW], BF16, tag="x", name=f"x_{c}")
            nc.sync.dma_start(xt[:], xC[ts(c, P), :])
            xts.append(xt)
        xt = xts[c]

        pss = [
            [
                ps_pool.tile([P, 512], F32, tag=f"ps{j}{h}", name=f"ps{j}{h}_{c}")
                for h in range(NH)
            ]
            for j in range(SUB)
        ]
        for k in range(K_TILES):
            for j in range(SUB):
                base = k * CHUNK + j * P
                for h in range(NH):
                    nc.tensor.matmul(
                        pss[j][h][:],
                        lhsT=xt[:, base : base + P],
                        rhs=ws[k][:, ts(h, 512)],
                        start=(k == 0),
                        stop=(k == K_TILES - 1),
                    )
        for j in range(SUB):
            o_sb = o_pool.tile([P, D_OUT], BF16, tag=f"o{j}", name=f"o{j}_{c}")
            # drains split ACT (h=0) / DVE (h=1); each half ships in its
            # own out-DMA (scalar / sync) so the final drain after the
            # last matmul is two parallel 128 KB transfers, not one
            # serialized 256 KB one.
            row = ts(c * SUB + j, P)
            nc.scalar.copy(o_sb[:, ts(0, 512)], pss[j][0][:])
            nc.vector.tensor_copy(o_sb[:, ts(1, 512)], pss[j][1][:])
            nc.scalar.dma_start(out[row, ts(0, 512)], o_sb[:, ts(0, 512)])
            nc.sync.dma_start(out[row, ts(1, 512)], o_sb[:, ts(1, 512)])


def build_nc() -> "bass.Bass":
    nc = bacc.Bacc(
        "TRN2",
        target_bir_lowering=False,
        debug=False,
        num_devices=N_CORES,
    )
    xC = nc.dram_tensor(
        "xC", [N_CHUNKS * P, XROW], BF16, kind="ExternalInput"
    ).ap()
    wsT = nc.dram_tensor("wsT", [D_IN, D_OUT], BF16, kind="ExternalInput").ap()
    out = nc.dram_tensor("out", [M_CORE, D_OUT], BF16, kind="ExternalOutput").ap()

    with tile.TileContext(nc) as tc, ExitStack() as ctx:
        _kernel_body(ctx, tc, xC, wsT, out)
    nc.compile()
    return nc


_NC_CACHE: list = []


def get_nc() -> "bass.Bass":
    if not _NC_CACHE:
        _NC_CACHE.append(build_nc())
    return _NC_CACHE[0]


def make_in_maps(x, weight, a_w, b_w, magnitude):
    # accept jax arrays / non-contiguous inputs from any harness
    x = np.asarray(x, dtype=np.float32)
    weight = np.asarray(weight, dtype=np.float32)
    a_w = np.asarray(a_w, dtype=np.float32)
    b_w = np.asarray(b_w, dtype=np.float32)
    magnitude = np.asarray(magnitude, dtype=np.float32)
    w_eff = weight + np.float32(SCALING) * (b_w @ a_w)
    norm = np.sqrt((w_eff.astype(np.float64) ** 2).sum(axis=1))
    s = (magnitude.astype(np.float64).reshape(-1) / norm).astype(np.float32)
    wsT = np.ascontiguousarray((w_eff * s[:, None]).T).astype(BF16_NP)

    # per-chunk SBUF layout: row c*128+p, col k*256+t  <-  x[core, c*256+t, k*128+p]
    xb = x.reshape(N_CORES, N_CHUNKS, CHUNK, K_TILES, P).astype(BF16_NP)
    xC = np.ascontiguousarray(np.transpose(xb, (0, 1, 4, 3, 2))).reshape(
        N_CORES, N_CHUNKS * P, XROW
    )
    return [{"xC": xC[i], "wsT": wsT} for i in range(N_CORES)]


def kernel(x, weight, a_w, b_w, magnitude):
    nc = get_nc()
    in_maps = make_in_maps(x, weight, a_w, b_w, magnitude)
    trace = os.environ.get("KERNEL_TRACE", "0") == "1"
    res = run_bass_kernel_spmd(nc, in_maps, list(range(N_CORES)), trace=trace)
    if trace:
        kernel.last_result = res
    outs = [res.results[i]["out"] for i in range(N_CORES)]
    return (
        np.concatenate(outs, axis=0).astype(np.float32).reshape(B, S, D_OUT)
    )
